# revision 39
# baseline (speedup 1.0000x reference)
"""Multi-head attention layer (T=1024, B=8, D=1024, H=16) on 8 TRN2 NeuronCores.

Sharding: data-parallel over batch B=8 -- one batch element per core, no
collectives. Each core computes its full attention layer slice:
  q/k/v projections -> causal softmax attention -> output projection.

Compute in bf16 on the TensorEngine (f32 PSUM accumulation); exp on ScalarE
fused with the PSUM eviction; softmax normalization via a ones-matmul column
reduction + fast approximate reciprocal fused into the output eviction.

Layouts (per core, host-side pre-transposed so every DMA is contiguous):
  xf/kf/vf: [D, T]  (X^T etc)       wq/wk/wv: [D, HN]    wo: [HN, D]
  On chip: QT/KT as [hn, t] (pairs of heads per 128 partitions), V as [t, hn].
  Scores computed transposed: ST[s, t] = KT_h.T-slice @ QT_h, 2-head
  row-packed (K=64). attn@V and Z (softmax denominator) 2-head col-packed.
"""

import numpy as np
import ml_dtypes

import concourse.bass as bass  # noqa: F401  (registers engine builders)
import concourse.bacc as bacc
import concourse.tile as tile
import concourse.mybir as mybir
from concourse.bass_utils import run_bass_kernel_spmd

T, B, D, H = 1024, 8, 1024, 16
NH = D // H          # 64 per-head width
P = 128              # SBUF partitions
NPAIR = H // 2       # 8 head-pairs (2 heads per 128 psum partitions)
DC = D // P          # 8 contraction chunks (bf16)
TT = T // P          # 8 t-tiles of 128
NCHUNK = 512         # matmul moving free dim / psum bank width (f32)
TC = T // NCHUNK     # 2 t-chunks
BF16 = mybir.dt.bfloat16
F32 = mybir.dt.float32
Exp = mybir.ActivationFunctionType.Exp
Copy = mybir.ActivationFunctionType.Copy
SCALE = 0.125  # 1/sqrt(NH)

N_CORES = 8
BF16_NP = ml_dtypes.bfloat16


def build_body(nc, tc, d, reps=1, causal=True, no_bias=False):
    """Emit the kernel body. d: dict of dram tensor handles."""
    import contextlib

    with contextlib.ExitStack() as ctx:
        sb_in = ctx.enter_context(tc.tile_pool(name="sb_in", bufs=2))
        sb_w = ctx.enter_context(tc.tile_pool(name="sb_w", bufs=2))
        sb_qkv = ctx.enter_context(tc.tile_pool(name="sb_qkv", bufs=1))
        sb_small = ctx.enter_context(tc.tile_pool(name="sb_small", bufs=1))
        sb_e = ctx.enter_context(tc.tile_pool(name="sb_e", bufs=10))
        sb_z = ctx.enter_context(tc.tile_pool(name="sb_z", bufs=2))
        sb_o = ctx.enter_context(tc.tile_pool(name="sb_o", bufs=4))
        ps = ctx.enter_context(tc.tile_pool(name="ps", bufs=1, space="PSUM"))

        # constants (loaded once)
        tri_t = sb_small.tile([P, P], BF16, tag="tri")
        ones64_t = sb_small.tile([P, NH], BF16, tag="ones64")
        ones1_t = sb_small.tile([1, P], BF16, tag="ones1")
        bqc_t = sb_small.tile([P, NPAIR], F32, tag="bqc")  # bq as [hn%128, pair]
        bkc_t = sb_small.tile([P, NPAIR], F32, tag="bkc")
        bv_t = sb_small.tile([1, D], BF16, tag="bv")
        bo_t = sb_small.tile([1, D], BF16, tag="bo")
        nc.sync.dma_start(out=tri_t[:], in_=d["tri"][:, :])
        nc.sync.dma_start(out=ones64_t[:], in_=d["ones64"][:, :])
        nc.sync.dma_start(out=ones1_t[:], in_=d["ones1"][:, :])
        nc.sync.dma_start(out=bqc_t[:], in_=d["bqc"][:, :])
        nc.sync.dma_start(out=bkc_t[:], in_=d["bkc"][:, :])
        nc.sync.dma_start(out=bv_t[:], in_=d["bv"][:, :])
        nc.sync.dma_start(out=bo_t[:], in_=d["bo"][:, :])

        for _ in range(reps):
            # ---- load inputs/weights (tag-shared slots rotate per phase) ----
            # per-chunk DMAs so the first matmuls only wait on their own chunk
            def load_mat(dram, tag):
                t_ = sb_in.tile([P, DC, T], BF16, tag=tag, bufs=1)
                src = dram.ap().rearrange("(c p) t -> p c t", p=P)
                for r in range(DC):
                    nc.sync.dma_start(out=t_[:, r, :], in_=src[:, r, :])
                return t_

            xf_t = load_mat(d["xf"], "xf")
            kf_t = load_mat(d["kf"], "kf")
            vf_t = load_mat(d["vf"], "vf")
            wq_t = sb_w.tile([P, DC, D], BF16, tag="wqk")
            wk_t = sb_w.tile([P, DC, D], BF16, tag="wqk")
            wv_t = sb_w.tile([P, DC, D], BF16, tag="wv", bufs=1)
            # wo rotates into wq's slot once the last QT matmul has consumed it
            wo_t = sb_w.tile([P, DC, D], BF16, tag="wqk")
            for name, t_ in (("wq", wq_t), ("wk", wk_t), ("wv", wv_t), ("wo", wo_t)):
                src = d[name].ap().rearrange("(c p) n -> p c n", p=P)
                for r in range(DC):
                    nc.sync.dma_start(out=t_[:, r, :], in_=src[:, r, :])

            qt = sb_qkv.tile([P, NPAIR, T], BF16, tag="qt")
            kt = sb_qkv.tile([P, NPAIR, T], BF16, tag="kt")
            vt = sb_qkv.tile([P, TT, D], BF16, tag="vt")
            ots = sb_qkv.tile([P, NPAIR, T], BF16, tag="ots")

            # ---- per head-pair: QT/KT/V projections + attention ----
            # Projections for pair q+1 are emitted as closures interleaved
            # one-per-j into pair q's attention loop: the ~9 projection
            # matmuls give the PE fill work while ScalarE runs exp, instead
            # of stalling at the attn@V matmul.
            def proj_units(q):
                hn0 = q * P
                units = []
                for dst, w_t, in_t, b_t in (
                    (qt, wq_t, xf_t, bqc_t), (kt, wk_t, kf_t, bkc_t)
                ):
                    for c in range(TC):
                        def f_qk(dst=dst, w_t=w_t, in_t=in_t, b_t=b_t,
                                 c=c, hn0=hn0, q=q):
                            t0 = c * NCHUNK
                            pj = ps.tile([P, NCHUNK], F32, tag="mm4", bufs=2)
                            for r in range(DC):
                                nc.tensor.matmul(
                                    pj[:],
                                    w_t[:, r, hn0 : hn0 + P],
                                    in_t[:, r, t0 : t0 + NCHUNK],
                                    start=(r == 0),
                                    stop=(r == DC - 1),
                                )
                            if no_bias:
                                nc.vector.tensor_copy(
                                    dst[:, q, t0 : t0 + NCHUNK], pj[:])
                            else:
                                # bias folded into eviction (per-part scalar)
                                nc.vector.tensor_scalar_add(
                                    dst[:, q, t0 : t0 + NCHUNK], pj[:],
                                    b_t[:, q : q + 1])
                        units.append(f_qk)
                for i in range(TT):
                    def f_v(i=i, hn0=hn0, q=q):
                        tt0 = i * P
                        pj = ps.tile([P, P], F32, tag="mm4", bufs=2)
                        for r in range(DC):
                            nc.tensor.matmul(
                                pj[:],
                                vf_t[:, r, tt0 : tt0 + P],
                                wv_t[:, r, hn0 : hn0 + P],
                                start=(r == 0),
                                stop=(r == DC - 1) and no_bias,
                            )
                        if not no_bias:
                            nc.tensor.matmul(
                                pj[:],
                                ones1_t[0:1, :],
                                bv_t[0:1, hn0 : hn0 + P],
                                start=False,
                                stop=True,
                            )
                        nc.vector.tensor_copy(vt[:, i, hn0 : hn0 + P], pj[:])
                    units.append(f_v)
                return units

            for f in proj_units(0):
                f()
            for q in range(NPAIR):
                h1c = (2 * q) * NH      # head1 col offset in vt
                h2c = (2 * q + 1) * NH
                filler = proj_units(q + 1) if q + 1 < NPAIR else []

                # attention for this pair, scores pipelined one j ahead
                for c in range(TC):
                    t0 = c * NCHUNK
                    jmax = 4 * (c + 1) if causal else TT
                    ot_ps = ps.tile([P, NCHUNK], F32, tag="ot", bufs=1)
                    z_ps = ps.tile([P, NCHUNK], F32, tag="z", bufs=1)

                    def offn(j):
                        off = max(0, j * P - t0) if causal else 0
                        return off, NCHUNK - off

                    def compute_st(j):
                        off, n = offn(j)
                        s0 = j * P
                        st1 = ps.tile([P, NCHUNK], F32, tag="st", bufs=4)
                        st2 = ps.tile([P, NCHUNK], F32, tag="st", bufs=4)
                        nc.tensor.matmul(
                            st1[:, :n],
                            kt[0:64, q, s0 : s0 + P],
                            qt[0:64, q, t0 + off : t0 + NCHUNK],
                            start=True, stop=True, tile_position=(0, 0),
                        )
                        nc.tensor.matmul(
                            st2[:, :n],
                            kt[64:128, q, s0 : s0 + P],
                            qt[64:128, q, t0 + off : t0 + NCHUNK],
                            start=True, stop=True, tile_position=(64, 0),
                        )
                        return st1, st2

                    sts = [compute_st(0)]
                    for j in range(jmax):
                        off, n = offn(j)
                        s0 = j * P
                        first, last = (j == 0), (j == jmax - 1)
                        st1, st2 = sts[j]
                        if j + 1 < jmax:
                            sts.append(compute_st(j + 1))
                        if filler:
                            filler.pop(0)()   # next pair's projection unit
                        e12 = sb_e.tile([P, 2 * NCHUNK], BF16, tag="e12")
                        nc.scalar.activation(
                            e12[:, :n], st1[:, :n], Exp, scale=SCALE)
                        nc.scalar.activation(
                            e12[:, NCHUNK : NCHUNK + n],
                            st2[:, :n], Exp, scale=SCALE)
                        if causal and s0 >= t0:
                            # diagonal tile: keep s <= t; multiplicative 0/1
                            # mask post-exp on the otherwise idle GpSimd
                            nc.gpsimd.tensor_mul(e12[:, 0:P], e12[:, 0:P], tri_t[:])
                            nc.gpsimd.tensor_mul(
                                e12[:, NCHUNK : NCHUNK + P],
                                e12[:, NCHUNK : NCHUNK + P], tri_t[:])
                        # attn @ V (2-head col-packed, M=64). Keep col
                        # positions ALTERNATING (0/64/0/64): back-to-back
                        # matmuls in the same col group stall the next
                        # LDWEIGHTS against the in-flight stream (grouping
                        # same-col emission measured +12 us on HW).
                        nc.tensor.matmul(
                            ot_ps[0:64, off:], vt[:, j, h1c : h1c + NH],
                            e12[:, :n],
                            start=first, stop=last, tile_position=(0, 0),
                            skip_group_check=True,
                        )
                        nc.tensor.matmul(
                            ot_ps[64:128, off:], vt[:, j, h2c : h2c + NH],
                            e12[:, NCHUNK : NCHUNK + n],
                            start=first, stop=last, tile_position=(0, 64),
                            skip_group_check=True,
                        )
                        # Z (softmax denominator, broadcast to 64 rows)
                        nc.tensor.matmul(
                            z_ps[0:64, off:], ones64_t[:], e12[:, :n],
                            start=first, stop=last, tile_position=(0, 0),
                            skip_group_check=True,
                        )
                        nc.tensor.matmul(
                            z_ps[64:128, off:], ones64_t[:],
                            e12[:, NCHUNK : NCHUNK + n],
                            start=first, stop=last, tile_position=(0, 64),
                            skip_group_check=True,
                        )
                    zinv = sb_z.tile([P, NCHUNK], F32, tag="zinv")
                    nc.vector.reciprocal_approx_fast(out=zinv[:], in_=z_ps[:])
                    nc.vector.tensor_mul(ots[:, q, t0 : t0 + NCHUNK], ot_ps[:], zinv[:])
                for f in filler:  # leftovers (non-causal has more js than units)
                    f()

            # ---- output projection ----
            for i in range(TT):
                tt0 = i * P
                for c in range(TC):
                    n0 = c * NCHUNK
                    po = ps.tile([P, NCHUNK], F32, tag="mm4", bufs=2)
                    for q in range(NPAIR):
                        nc.tensor.matmul(
                            po[:],
                            ots[:, q, tt0 : tt0 + P],
                            wo_t[:, q, n0 : n0 + NCHUNK],
                            start=(q == 0),
                            stop=(q == NPAIR - 1) and no_bias,
                        )
                    if not no_bias:
                        nc.tensor.matmul(
                            po[:],
                            ones1_t[0:1, :],
                            bo_t[0:1, n0 : n0 + NCHUNK],
                            start=False,
                            stop=True,
                        )
                    so = sb_o.tile([P, NCHUNK], F32, tag="so")
                    nc.scalar.activation(so[:], po[:], Copy)
                    nc.sync.dma_start(
                        out=d["out"][tt0 : tt0 + P, n0 : n0 + NCHUNK], in_=so[:]
                    )


def build(causal=True, reps=1, no_bias=False):
    nc = bacc.Bacc("TRN2", target_bir_lowering=False, debug=False,
                   num_devices=N_CORES)
    d = {}
    for name in ("xf", "kf", "vf"):
        d[name] = nc.dram_tensor(name, [D, T], BF16, kind="ExternalInput")
    for name in ("wq", "wk", "wv", "wo"):
        d[name] = nc.dram_tensor(name, [D, D], BF16, kind="ExternalInput")
    for name in ("bv", "bo"):
        d[name] = nc.dram_tensor(name, [1, D], BF16, kind="ExternalInput")
    for name in ("bqc", "bkc"):
        d[name] = nc.dram_tensor(name, [P, NPAIR], F32, kind="ExternalInput")
    d["tri"] = nc.dram_tensor("tri", [P, P], BF16, kind="ExternalInput")
    d["ones64"] = nc.dram_tensor("ones64", [P, NH], BF16, kind="ExternalInput")
    d["ones1"] = nc.dram_tensor("ones1", [1, P], BF16, kind="ExternalInput")
    d["out"] = nc.dram_tensor("out", [T, D], F32, kind="ExternalOutput")

    with tile.TileContext(nc) as tc:
        build_body(nc, tc, d, reps=reps, causal=causal, no_bias=no_bias)
    nc.compile()
    return nc


def make_in_maps(input_tensor, keys_vector, values_vector, Wq, bq, Wk, bk,
                 Wv, bv, Wo, bo):
    """Host-side sharding + layout transforms + bf16 casts."""
    def b16(a):
        return np.ascontiguousarray(a).astype(BF16_NP)

    shared = {
        "wq": b16(np.asarray(Wq, np.float32).transpose(1, 0, 2).reshape(D, D)),
        "wk": b16(np.asarray(Wk, np.float32).transpose(1, 0, 2).reshape(D, D)),
        "wv": b16(np.asarray(Wv, np.float32).transpose(1, 0, 2).reshape(D, D)),
        "wo": b16(np.asarray(Wo, np.float32).T),
        # bq/bk as [hn % 128, pair] f32 columns for per-partition bias add
        "bqc": np.ascontiguousarray(
            np.asarray(bq, np.float32).reshape(NPAIR, P).T),
        "bkc": np.ascontiguousarray(
            np.asarray(bk, np.float32).reshape(NPAIR, P).T),
        "bv": b16(np.asarray(bv, np.float32).reshape(1, D)),
        "bo": b16(np.asarray(bo, np.float32).reshape(1, D)),
        "tri": np.triu(np.ones((P, P), np.float32)).astype(BF16_NP),
        "ones64": np.ones((P, NH), BF16_NP),
        "ones1": np.ones((1, P), BF16_NP),
    }
    x = np.asarray(input_tensor, np.float32)
    k = np.asarray(keys_vector, np.float32)
    v = np.asarray(values_vector, np.float32)
    in_maps = []
    for c in range(N_CORES):
        m = dict(shared)
        m["xf"] = b16(x[:, c, :].T)
        m["kf"] = b16(k[:, c, :].T)
        m["vf"] = b16(v[:, c, :].T)
        in_maps.append(m)
    return in_maps


_NC_CACHE = {}


def kernel(input_tensor, keys_vector, values_vector, Wq, bq, Wk, bk, Wv, bv,
           Wo, bo, mask):
    causal = bool(int(np.asarray(mask)))
    no_bias = all(
        not np.any(np.asarray(b)) for b in (bq, bk, bv, bo)
    )
    key = (causal, no_bias)
    if key not in _NC_CACHE:
        _NC_CACHE[key] = build(causal=causal, no_bias=no_bias)
    nc = _NC_CACHE[key]
    in_maps = make_in_maps(input_tensor, keys_vector, values_vector, Wq, bq,
                           Wk, bk, Wv, bv, Wo, bo)
    res = run_bass_kernel_spmd(nc, in_maps, core_ids=list(range(N_CORES)))
    out = np.empty((T, B, D), np.float32)
    for c in range(N_CORES):
        out[:, c, :] = res.results[c]["out"]
    return out
